# revision 5
# baseline (speedup 1.0000x reference)
"""ABMIL attention pooling on 8 TRN2 NeuronCores.

Algorithm (per bag b):
    a_n   = tanh(x_n . w1) * sigmoid(x_n . w2)            (gated attention score)
    att   = softmax over valid n of a                     (masked)
    out_b = sum_n att_n * (x_n . wf_l)                    (fold wf into the score matmul)

Key algebraic fold: out = (sum att_n x_n) @ wf.T == sum att_n (x_n @ wf.T),
so the only large compute is ONE matmul  y = xs @ [wf.T | w1 | w2]  ([N, 6]),
and the pooling reduces tiny [N, 6] data.  Scores lie in (-1, 1) so no
softmax max-subtraction is needed: p = exp(a) * valid, out = sum(p*y)/sum(p).

sigmoid(x) = 0.5*(1 + tanh(x/2)) keeps all transcendentals in one ACT table
set (tanh + exp).

Engine ops require partition bases == 0 (mod 32), so:
  - the matmul is split into two PSUM groups (wf -> [4,NT], w1w2 -> [2,NT]),
  - tanh(row1) is extracted to its own [1,NT] via a K=2 select-matmul,
  - e is broadcast to 4 partitions via a K=1 ones-matmul,
  - per-bag accumulators live at partitions 32*b.
float32r (full-rate fp32 PE mode, ~1e-4 rel precision) is used for all PE
streams; it must be produced typed (DRAM f32r inputs, ACT f32r outputs).

Sharding: instance dim N split 8 ways (2048 instances x 4 bags per core,
32 MiB of xs each).  The host pre-packs each shard transposed + tiled so
every device DMA is a plain contiguous [128, 4096] f32 load.  Each core
accumulates per-bag (sum p, sum p*y) partials, AllReduces the 20-float
stats across the 8 cores, and finalizes out = t/s on device.  Invalid
instances are zeroed in the packed xs (host) so their y rows are 0; the
valid mask only enters the softmax denominator.
"""

import numpy as np

B, N, D, L = 4, 16384, 1024, 4
NCORES = 8
NSH = N // NCORES            # 2048 instances per bag per core
J = B * NSH                  # 8192 flattened rows per core
NT = 512                     # instances per compute tile (1 PSUM bank)
T = J // NT                  # 16 tiles
C8 = D // 128                # 8 contraction chunks of 128
TPB = NSH // NT              # tiles per bag = 4

_NC_CACHE = {}


def _build_nc():
    from concourse import bacc, mybir, tile

    dt = mybir.dt
    act = mybir.ActivationFunctionType
    alu = mybir.AluOpType
    f32 = dt.float32
    f32r = dt.float32r

    nc = bacc.Bacc(
        "TRN2", target_bir_lowering=False, debug=False, num_devices=NCORES
    )

    xsp = nc.dram_tensor("xsp", [T, 128, C8 * NT], f32r, kind="ExternalInput").ap()
    # [128, 48] = [wf-packed (8 chunks x 4) | w1w2-packed (8 chunks x 2)]
    wsb = nc.dram_tensor("wsb", [128, C8 * 6], f32r, kind="ExternalInput").ap()
    mk = nc.dram_tensor("mk", [1, J], f32, kind="ExternalInput").ap()
    # cstr: col0 = select-row-1 = [0,1]; row0 cols1:5 = ones
    cstr = nc.dram_tensor("cstr", [2, 5], f32r, kind="ExternalInput").ap()
    # cstf: per-partition tanh input scales [1.0, 0.5]
    cstf = nc.dram_tensor("cstf", [2, 1], f32, kind="ExternalInput").ap()
    outp = nc.dram_tensor("out", [B, L], f32, kind="ExternalOutput").ap()

    with tile.TileContext(nc) as tc:
        with (
            tc.tile_pool(name="const", bufs=1) as constp,
            tc.tile_pool(name="xs", bufs=3) as xpool,
            tc.tile_pool(name="psA", bufs=2, space="PSUM") as apool,
            tc.tile_pool(name="psB", bufs=2, space="PSUM") as bpool,
            tc.tile_pool(name="psS1", bufs=2, space="PSUM") as s1pool,
            tc.tile_pool(name="psS2", bufs=2, space="PSUM") as s2pool,
            tc.tile_pool(name="sm", bufs=3) as smp,
            tc.tile_pool(name="dram", bufs=1, space="DRAM") as dramp,
        ):
            w_sb = constp.tile([128, C8 * 6], f32r, tag="w")
            nc.sync.dma_start(w_sb[:], wsb)
            m_sb = constp.tile([1, J], f32, tag="m")
            nc.sync.dma_start(m_sb[:], mk)
            cr = constp.tile([2, 5], f32r, tag="cr")
            nc.sync.dma_start(cr[:], cstr)
            cf = constp.tile([2, 1], f32, tag="cf")
            nc.sync.dma_start(cf[:], cstf)
            accPs, accYs = [], []
            for b in range(B):
                aP = constp.tile([1, NT], f32, tag=f"accP{b}")
                nc.vector.memset(aP[:], 0.0)
                accPs.append(aP)
                aY = constp.tile([4, NT], f32, tag=f"accY{b}")
                nc.vector.memset(aY[:], 0.0)
                accYs.append(aY)

            sel = cr[0:2, 0:1]
            ones4 = cr[0:1, 1:5]

            for t in range(T):
                bg = t // TPB  # bag index for this tile
                xt = xpool.tile([128, C8 * NT], f32r, tag="xt")
                nc.sync.dma_start(xt[:], xsp[t])

                # psA[0:4, n] = wf @ x ; psB[0:2, n] = [w1 | w2].T @ x
                psA = apool.tile([4, NT], f32, tag="yA")
                psB = bpool.tile([2, NT], f32, tag="yB")
                for c in range(C8):
                    nc.tensor.matmul(
                        psA[:],
                        w_sb[:, c * 4 : (c + 1) * 4],
                        xt[:, c * NT : (c + 1) * NT],
                        start=(c == 0),
                        stop=(c == C8 - 1),
                    )
                for c in range(C8):
                    nc.tensor.matmul(
                        psB[:],
                        w_sb[:, 32 + c * 2 : 32 + (c + 1) * 2],
                        xt[:, c * NT : (c + 1) * NT],
                        start=(c == 0),
                        stop=(c == C8 - 1),
                    )

                # tts = [tanh(s1), tanh(s2/2)]  (per-partition scale [1, .5])
                tts = smp.tile([2, NT], f32r, tag="tts")
                nc.scalar.activation(tts[:], psB[:], act.Tanh, scale=cf[:])
                # extract row 1 to its own base-0 tile via select-matmul
                t1x = s1pool.tile([1, NT], f32, tag="t1x")
                nc.tensor.matmul(t1x[:], sel, tts[:], start=True, stop=True)
                # v = (tanh(s2/2) + 1) * tanh(s1)  == 2*a ;  e = exp(v/2)
                v = smp.tile([1, NT], f32, tag="v")
                nc.vector.scalar_tensor_tensor(
                    v[:], t1x[:], 1.0, tts[0:1, :].bitcast(f32), alu.add, alu.mult
                )
                e = smp.tile([1, NT], f32r, tag="e")
                nc.scalar.activation(e[:], v[:], act.Exp, scale=0.5)

                # broadcast e to 4 partitions via K=1 ones-matmul
                ebp = s2pool.tile([4, NT], f32, tag="eb")
                nc.tensor.matmul(ebp[:], ones4, e[:], start=True, stop=True)
                ebs = smp.tile([4, NT], f32, tag="ebs")
                nc.scalar.copy(ebs[:], ebp[:])

                # p = e * mask ; py = y * e  (invalid rows of y are zero)
                tmpP = smp.tile([1, NT], f32, tag="tmpP")
                nc.vector.tensor_tensor(
                    tmpP[:],
                    e[:].bitcast(f32),
                    m_sb[0:1, t * NT : (t + 1) * NT],
                    alu.mult,
                )
                tmpY = smp.tile([4, NT], f32, tag="tmpY")
                nc.vector.tensor_tensor(tmpY[:], psA[:], ebs[:], alu.mult)
                nc.vector.tensor_tensor(accPs[bg][:], accPs[bg][:], tmpP[:], alu.add)
                nc.vector.tensor_tensor(accYs[bg][:], accYs[bg][:], tmpY[:], alu.add)

            # free-dim reduce -> per-bag scalars, then gather into DRAM
            redP = constp.tile([1, B], f32, tag="redP")
            redY = constp.tile([4, B], f32, tag="redY")
            for b in range(B):
                nc.vector.tensor_reduce(
                    redP[0:1, b : b + 1], accPs[b][:], mybir.AxisListType.X, alu.add
                )
                nc.vector.tensor_reduce(
                    redY[0:4, b : b + 1], accYs[b][:], mybir.AxisListType.X, alu.add
                )

            # [s_b, t_b0..t_b3] per bag
            cc_in = dramp.tile([5 * B, 1], f32, tag="cc_in")
            cc_out = dramp.tile([5 * B, 1], f32, tag="cc_out")
            for b in range(B):
                nc.sync.dma_start(cc_in[5 * b : 5 * b + 1], redP[0:1, b : b + 1])
                nc.sync.dma_start(
                    cc_in[5 * b + 1 : 5 * b + 5], redY[0:4, b : b + 1]
                )
            nc.gpsimd.collective_compute(
                "AllReduce",
                alu.add,
                replica_groups=[list(range(NCORES))],
                ins=[cc_in[:].opt()],
                outs=[cc_out[:].opt()],
            )
            redsb = constp.tile([1, 5 * B], f32, tag="redsb")
            nc.sync.dma_start(
                redsb[:], cc_out[:].rearrange("p x -> (p x)").unsqueeze(0)
            )

            # out[b, l] = t_{b,l} / s_b
            rec = constp.tile([1, 5 * B], f32, tag="rec")
            nc.vector.reciprocal(rec[:], redsb[:])
            fin = constp.tile([1, B * L], f32, tag="fin")
            tv = redsb[:].rearrange("p (b j) -> p b j", b=B)[:, :, 1 : 1 + L]
            rv = (
                rec[:]
                .rearrange("p (b j) -> p b j", b=B)[:, :, 0:1]
                .broadcast_to([1, B, L])
            )
            nc.vector.tensor_tensor(
                fin[:].rearrange("p (b l) -> p b l", b=B), tv, rv, alu.mult
            )
            nc.sync.dma_start(outp.rearrange("b l -> (b l)").unsqueeze(0), fin[:])

    nc.compile()
    return nc


def _get_nc():
    if "nc" not in _NC_CACHE:
        _NC_CACHE["nc"] = _build_nc()
    return _NC_CACHE["nc"]


def _make_in_maps(xs, valid, w1, w2, wf):
    validf = valid.astype(np.float32)
    xsz = (xs.astype(np.float32) * validf[..., None]).astype(np.float32)
    # wf block: [128, 32] with (p, c*4+l) = wf[l, c*128+p]
    wA = np.ascontiguousarray(
        wf.astype(np.float32).T.reshape(C8, 128, L).transpose(1, 0, 2).reshape(128, C8 * L)
    )
    # w1w2 block: [128, 16] with (p, c*2+j) = [w1|w2][c*128+p, j]
    w12 = np.concatenate([w1.astype(np.float32), w2.astype(np.float32)], axis=1)
    wB = np.ascontiguousarray(
        w12.reshape(C8, 128, 2).transpose(1, 0, 2).reshape(128, C8 * 2)
    )
    wsb = np.ascontiguousarray(np.concatenate([wA, wB], axis=1))

    cstr = np.zeros((2, 5), np.float32)
    cstr[1, 0] = 1.0  # select row 1
    cstr[0, 1:5] = 1.0  # ones for e-broadcast
    cstf = np.array([[1.0], [0.5]], np.float32)

    in_maps = []
    for c in range(NCORES):
        sh = xsz[:, c * NSH : (c + 1) * NSH, :].reshape(J, D)
        xt = sh.T  # [D, J]
        packed = np.ascontiguousarray(
            xt.reshape(C8, 128, T, NT).transpose(2, 1, 0, 3)
        ).reshape(T, 128, C8 * NT)
        mkc = np.ascontiguousarray(validf[:, c * NSH : (c + 1) * NSH].reshape(1, J))
        in_maps.append(
            {"xsp": packed, "wsb": wsb, "mk": mkc, "cstr": cstr, "cstf": cstf}
        )
    return in_maps


def _run(xs, valid, w1, w2, wf, trace=False, **kwargs):
    from concourse import bass_utils

    nc = _get_nc()
    in_maps = _make_in_maps(xs, valid, w1, w2, wf)
    res = bass_utils.run_bass_kernel_spmd(
        nc, in_maps, core_ids=list(range(NCORES)), trace=trace, **kwargs
    )
    return res


def kernel(xs, valid, w1, w2, wf):
    res = _run(xs, valid, w1, w2, wf, trace=False)
    return np.asarray(res.results[0]["out"]).astype(np.float32)


# revision 6
# speedup vs baseline: 1.3047x; 1.3047x over previous
"""ABMIL attention pooling on 8 TRN2 NeuronCores.

Algorithm (per bag b):
    a_n   = tanh(x_n . w1) * sigmoid(x_n . w2)            (gated attention score)
    att   = softmax over valid n of a                     (masked)
    out_b = sum_n att_n * (x_n . wf_l)                    (fold wf into the score matmul)

Key algebraic fold: out = (sum att_n x_n) @ wf.T == sum att_n (x_n @ wf.T),
so the only large compute is ONE matmul  y = xs @ [wf.T | w1 | w2]  ([N, 6])
and the pooling reduces tiny [N, 6] data.  Scores lie in (-1, 1) so no
softmax max-subtraction is needed: p = exp(a) * valid, out = sum(p*y)/sum(p).

sigmoid(x) = 0.5*(1 + tanh(x/2)) keeps all transcendentals in one ACT table
set (tanh + exp).

Per-core pipeline (memory-bound target: 32 MiB of xs @ ~355 GB/s ~ 95 us):
  - xs tiles stream in with an f32->bf16 cast-DMA (SWDGE, measured at line
    rate), so the PE runs bf16 at 1 col/cycle.
  - ONE 6-wide accumulation matmul group per 512-instance tile:
    psY[0:6, n] = [wf | w1 | w2].T @ x  (8 chunk matmuls over D=1024).
  - Engine ops need partition bases == 0 mod 32, so tanh runs on all 6 rows
    (rows 0..3 are don't-care) with per-partition scale [1,1,1,1,1,0.5], and
    the two score rows are extracted to base-0 PSUM tiles via K=6
    select-matmuls; e is broadcast to 4 partitions via a K=1 ones-matmul.
  - Per-bag accumulators (sum p, sum p*y) are separate base-0 tiles; final
    20-float stats are AllReduced across the 8 cores and out = t/s is
    computed on device (identical on every core).

Invalid instances are zeroed in the packed xs (host) so their y rows are 0;
the valid mask enters only the softmax denominator.
"""

import numpy as np

B, N, D, L = 4, 16384, 1024, 4
NCORES = 8
NSH = N // NCORES            # 2048 instances per bag per core
J = B * NSH                  # 8192 flattened rows per core
NT = 512                     # instances per compute tile (1 PSUM bank)
T = J // NT                  # 16 tiles
C8 = D // 128                # 8 contraction chunks of 128
TPB = NSH // NT              # tiles per bag = 4

_NC_CACHE = {}


def _build_nc():
    from concourse import bacc, mybir, tile

    dt = mybir.dt
    act = mybir.ActivationFunctionType
    alu = mybir.AluOpType
    f32 = dt.float32
    bf16 = dt.bfloat16

    nc = bacc.Bacc(
        "TRN2", target_bir_lowering=False, debug=False, num_devices=NCORES
    )

    xsp = nc.dram_tensor("xsp", [T, 128, C8 * NT], f32, kind="ExternalInput").ap()
    # [128, 48]: per chunk c, cols c*6..c*6+5 = [wf0..wf3, w1, w2]
    wsb = nc.dram_tensor("wsb", [128, C8 * 6], bf16, kind="ExternalInput").ap()
    mk = nc.dram_tensor("mk", [1, J], f32, kind="ExternalInput").ap()
    # cstr: col0 = select-row-4, col1 = select-row-5, row0 cols2:6 = ones
    cstr = nc.dram_tensor("cstr", [6, 6], bf16, kind="ExternalInput").ap()
    # cstf: per-partition tanh input scales [1,1,1,1,1,0.5]
    cstf = nc.dram_tensor("cstf", [6, 1], f32, kind="ExternalInput").ap()
    outp = nc.dram_tensor("out", [B, L], f32, kind="ExternalOutput").ap()

    with tile.TileContext(nc) as tc:
        with (
            tc.tile_pool(name="const", bufs=1) as constp,
            tc.tile_pool(name="xs", bufs=4) as xpool,
            tc.tile_pool(name="psY", bufs=3, space="PSUM") as ypool,
            tc.tile_pool(name="psTA", bufs=1, space="PSUM") as tapool,
            tc.tile_pool(name="psTB", bufs=1, space="PSUM") as tbpool,
            tc.tile_pool(name="psEB", bufs=2, space="PSUM") as ebpool,
            tc.tile_pool(name="sm", bufs=3) as smp,
            tc.tile_pool(name="dram", bufs=1, space="DRAM") as dramp,
        ):
            w_sb = constp.tile([128, C8 * 6], bf16, tag="w")
            nc.sync.dma_start(w_sb[:], wsb)
            m_sb = constp.tile([1, J], f32, tag="m")
            nc.sync.dma_start(m_sb[:], mk)
            cr = constp.tile([6, 6], bf16, tag="cr")
            nc.sync.dma_start(cr[:], cstr)
            cf = constp.tile([6, 1], f32, tag="cf")
            nc.sync.dma_start(cf[:], cstf)
            accPs, accYs = [], []
            for b in range(B):
                aP = constp.tile([1, NT], f32, tag=f"accP{b}")
                nc.vector.memset(aP[:], 0.0)
                accPs.append(aP)
                aY = constp.tile([4, NT], f32, tag=f"accY{b}")
                nc.vector.memset(aY[:], 0.0)
                accYs.append(aY)

            selA = cr[0:6, 0:1]
            selB = cr[0:6, 1:2]
            ones4 = cr[0:1, 2:6]

            for t in range(T):
                bg = t // TPB  # bag index for this tile
                xt = xpool.tile([128, C8 * NT], bf16, tag="xt")
                nc.gpsimd.dma_start(xt[:], xsp[t])  # f32 -> bf16 cast DMA

                # psY[0:6, n] = [wf | w1 | w2].T @ x  (accumulate over d)
                psY = ypool.tile([6, NT], f32, tag="y")
                for c in range(C8):
                    nc.tensor.matmul(
                        psY[:],
                        w_sb[:, c * 6 : (c + 1) * 6],
                        xt[:, c * NT : (c + 1) * NT],
                        start=(c == 0),
                        stop=(c == C8 - 1),
                    )

                # tanh of all 6 rows (only rows 4,5 used), row5 pre-scaled .5
                tts = smp.tile([6, NT], bf16, tag="tts")
                nc.scalar.activation(tts[:], psY[:], act.Tanh, scale=cf[:])
                # extract tanh(s1), tanh(s2/2) to base-0 PSUM tiles
                tA = tapool.tile([1, NT], f32, tag="tA")
                nc.tensor.matmul(tA[:], selA, tts[:], start=True, stop=True)
                tB = tbpool.tile([1, NT], f32, tag="tB")
                nc.tensor.matmul(tB[:], selB, tts[:], start=True, stop=True)

                # v = tanh(s1) * (tanh(s2/2) + 1) == 2*a ;  e = exp(v/2)
                v1 = smp.tile([1, NT], f32, tag="v1")
                nc.vector.tensor_scalar_add(v1[:], tB[:], 1.0)
                v = smp.tile([1, NT], f32, tag="v")
                nc.vector.tensor_tensor(v[:], tA[:], v1[:], alu.mult)
                e = smp.tile([1, NT], bf16, tag="e")
                nc.scalar.activation(e[:], v[:], act.Exp, scale=0.5)

                # broadcast e to 4 partitions via K=1 ones-matmul
                ebp = ebpool.tile([4, NT], f32, tag="eb")
                nc.tensor.matmul(ebp[:], ones4, e[:], start=True, stop=True)
                ebs = smp.tile([4, NT], f32, tag="ebs")
                nc.scalar.copy(ebs[:], ebp[:])

                # p = e * mask ; py = y * e  (invalid rows of y are zero)
                tmpP = smp.tile([1, NT], f32, tag="tmpP")
                nc.vector.tensor_tensor(
                    tmpP[:], e[:], m_sb[0:1, t * NT : (t + 1) * NT], alu.mult
                )
                tmpY = smp.tile([4, NT], f32, tag="tmpY")
                nc.vector.tensor_tensor(tmpY[:], psY[0:4, :], ebs[:], alu.mult)
                nc.vector.tensor_tensor(accPs[bg][:], accPs[bg][:], tmpP[:], alu.add)
                nc.vector.tensor_tensor(accYs[bg][:], accYs[bg][:], tmpY[:], alu.add)

            # free-dim reduce -> per-bag scalars, gather to DRAM
            redP = constp.tile([1, B], f32, tag="redP")
            redY = constp.tile([4, B], f32, tag="redY")
            for b in range(B):
                nc.vector.tensor_reduce(
                    redP[0:1, b : b + 1], accPs[b][:], mybir.AxisListType.X, alu.add
                )
                nc.vector.tensor_reduce(
                    redY[0:4, b : b + 1], accYs[b][:], mybir.AxisListType.X, alu.add
                )

            # cc layout: flat[0:4] = s_b ; flat[4 + l*4 + b] = t_{b,l}
            cc_in = dramp.tile([1, 5 * B], f32, tag="cc_in")
            cc_out = dramp.tile([1, 5 * B], f32, tag="cc_out")
            nc.sync.dma_start(cc_in[0:1, 0:B], redP[:])
            nc.sync.dma_start(
                cc_in[0:1, B : 5 * B].rearrange("p (l b) -> l (p b)", l=4), redY[:]
            )
            nc.gpsimd.collective_compute(
                "AllReduce",
                alu.add,
                replica_groups=[list(range(NCORES))],
                ins=[cc_in[:].opt()],
                outs=[cc_out[:].opt()],
            )
            redsb = constp.tile([1, 5 * B], f32, tag="redsb")
            nc.sync.dma_start(redsb[:], cc_out[:])

            # out[b, l] = t_{b,l} / s_b
            rec = constp.tile([1, 5 * B], f32, tag="rec")
            nc.vector.reciprocal(rec[:], redsb[:])
            fin = constp.tile([1, B * L], f32, tag="fin")
            tv = redsb[:, B : 5 * B].rearrange("p (l b) -> p b l", l=L)
            rv = rec[:, 0:B].unsqueeze(2).broadcast_to([1, B, L])
            nc.vector.tensor_tensor(
                fin[:].rearrange("p (b l) -> p b l", b=B), tv, rv, alu.mult
            )
            nc.sync.dma_start(outp.rearrange("b l -> (b l)").unsqueeze(0), fin[:])

    nc.compile()
    return nc


def _get_nc():
    if "nc" not in _NC_CACHE:
        _NC_CACHE["nc"] = _build_nc()
    return _NC_CACHE["nc"]


def _make_in_maps(xs, valid, w1, w2, wf):
    import ml_dtypes

    validf = valid.astype(np.float32)
    xsz = (xs.astype(np.float32) * validf[..., None]).astype(np.float32)
    # [D, 6] = [wf.T | w1 | w2], packed per 128-chunk: (p, c*6+j) = W6[c*128+p, j]
    W6 = np.concatenate(
        [wf.astype(np.float32).T, w1.astype(np.float32), w2.astype(np.float32)],
        axis=1,
    )
    wsb = np.ascontiguousarray(
        W6.reshape(C8, 128, 6).transpose(1, 0, 2).reshape(128, C8 * 6)
    ).astype(ml_dtypes.bfloat16)

    cstr = np.zeros((6, 6), np.float32)
    cstr[4, 0] = 1.0  # selA -> row 4 (tanh(s1))
    cstr[5, 1] = 1.0  # selB -> row 5 (tanh(s2/2))
    cstr[0, 2:6] = 1.0  # ones for e-broadcast
    cstr = cstr.astype(ml_dtypes.bfloat16)
    cstf = np.array([[1.0]] * 5 + [[0.5]], np.float32)

    in_maps = []
    for c in range(NCORES):
        sh = xsz[:, c * NSH : (c + 1) * NSH, :].reshape(J, D)
        xt = sh.T  # [D, J]
        packed = np.ascontiguousarray(
            xt.reshape(C8, 128, T, NT).transpose(2, 1, 0, 3)
        ).reshape(T, 128, C8 * NT)
        mkc = np.ascontiguousarray(validf[:, c * NSH : (c + 1) * NSH].reshape(1, J))
        in_maps.append(
            {"xsp": packed, "wsb": wsb, "mk": mkc, "cstr": cstr, "cstf": cstf}
        )
    return in_maps


def _run(xs, valid, w1, w2, wf, trace=False, **kwargs):
    from concourse import bass_utils

    nc = _get_nc()
    in_maps = _make_in_maps(xs, valid, w1, w2, wf)
    res = bass_utils.run_bass_kernel_spmd(
        nc, in_maps, core_ids=list(range(NCORES)), trace=trace, **kwargs
    )
    return res


def kernel(xs, valid, w1, w2, wf):
    res = _run(xs, valid, w1, w2, wf, trace=False)
    return np.asarray(res.results[0]["out"]).astype(np.float32)


# revision 8
# speedup vs baseline: 1.4131x; 1.0830x over previous
"""ABMIL attention pooling on 8 TRN2 NeuronCores.

Algorithm (per bag b):
    a_n   = tanh(x_n . w1) * sigmoid(x_n . w2)            (gated attention score)
    att   = softmax over valid n of a                     (masked)
    out_b = sum_n att_n * (x_n . wf_l)                    (fold wf into the score matmul)

Key algebraic fold: out = (sum att_n x_n) @ wf.T == sum att_n (x_n @ wf.T),
so the only large compute is ONE matmul  y = xs @ [wf.T | w1 | w2]  ([N, 6])
and the pooling reduces tiny [N, 6] data.  Scores lie in (-1, 1) so no
softmax max-subtraction is needed: p = exp(a) * valid, out = sum(p*y)/sum(p).

sigmoid(x) = 0.5*(1 + tanh(x/2)) keeps all transcendentals in one ACT table
set (tanh + exp).

Per-core pipeline (memory-bound target: 32 MiB of xs @ ~355 GB/s ~ 95 us):
  - xs tiles stream in with an f32->bf16 cast-DMA (SWDGE, measured at line
    rate), so the PE runs bf16 at 1 col/cycle.
  - ONE 6-wide accumulation matmul group per 512-instance tile:
    psY[0:6, n] = [wf | w1 | w2].T @ x  (8 chunk matmuls over D=1024).
  - Engine ops need partition bases == 0 mod 32, so tanh runs on all 6 rows
    (rows 0..3 are don't-care) with per-partition scale [1,1,1,1,1,0.5], and
    the two score rows are extracted to base-0 PSUM tiles via K=6
    select-matmuls; e is broadcast to 4 partitions via a K=1 ones-matmul.
  - Per-bag accumulators (sum p, sum p*y) are separate base-0 tiles; final
    20-float stats are AllReduced across the 8 cores and out = t/s is
    computed on device (identical on every core).

Invalid instances are zeroed in the packed xs (host) so their y rows are 0;
the valid mask enters only the softmax denominator.
"""

import numpy as np

B, N, D, L = 4, 16384, 1024, 4
NCORES = 8
NSH = N // NCORES            # 2048 instances per bag per core
J = B * NSH                  # 8192 flattened rows per core
NT = 512                     # instances per compute tile (1 PSUM bank)
T = J // NT                  # 16 tiles
C8 = D // 128                # 8 contraction chunks of 128
TPB = NSH // NT              # tiles per bag = 4

_NC_CACHE = {}


def _build_nc():
    from concourse import bacc, mybir, tile

    dt = mybir.dt
    act = mybir.ActivationFunctionType
    alu = mybir.AluOpType
    f32 = dt.float32
    bf16 = dt.bfloat16

    nc = bacc.Bacc(
        "TRN2", target_bir_lowering=False, debug=False, num_devices=NCORES
    )

    xsp = nc.dram_tensor("xsp", [T, 128, C8 * NT], f32, kind="ExternalInput").ap()
    # [128, 48]: per chunk c, cols c*6..c*6+5 = [wf0..wf3, w1, w2]
    wsb = nc.dram_tensor("wsb", [128, C8 * 6], bf16, kind="ExternalInput").ap()
    # additive mask bias: 0 for valid, -20000 for invalid (pre-tanh-product)
    mb = nc.dram_tensor("mb", [1, J], bf16, kind="ExternalInput").ap()
    # cstf: per-partition tanh input scales [1,1,1,1,1,0.5]
    cstf = nc.dram_tensor("cstf", [6, 1], f32, kind="ExternalInput").ap()
    outp = nc.dram_tensor("out", [B, L], f32, kind="ExternalOutput").ap()

    with tile.TileContext(nc) as tc:
        with (
            tc.tile_pool(name="const", bufs=1) as constp,
            tc.tile_pool(name="xs", bufs=4) as xpool,
            tc.tile_pool(name="psY", bufs=4, space="PSUM") as ypool,
            tc.tile_pool(name="sm", bufs=3) as smp,
            tc.tile_pool(name="dram", bufs=1, space="DRAM") as dramp,
        ):
            w_sb = constp.tile([128, C8 * 6], bf16, tag="w")
            nc.sync.dma_start(w_sb[:], wsb)
            m_sb = constp.tile([1, J], bf16, tag="m")
            nc.sync.dma_start(m_sb[:], mb)
            cf = constp.tile([6, 1], f32, tag="cf")
            nc.sync.dma_start(cf[:], cstf)
            sPP = constp.tile([1, T], f32, tag="sPP")
            sYY = constp.tile([4, T], f32, tag="sYY")

            for t in range(T):
                bg = t // TPB  # bag index for this tile
                xt = xpool.tile([128, C8 * NT], bf16, tag="xt")
                nc.gpsimd.dma_start(xt[:], xsp[t])  # f32 -> bf16 cast DMA

                # psY[0:6, n] = [wf | w1 | w2].T @ x  (accumulate over d)
                psY = ypool.tile([6, NT], f32, tag="y")
                for c in range(C8):
                    nc.tensor.matmul(
                        psY[:],
                        w_sb[:, c * 6 : (c + 1) * 6],
                        xt[:, c * NT : (c + 1) * NT],
                        start=(c == 0),
                        stop=(c == C8 - 1),
                    )

                # tanh of all 6 rows (only rows 4,5 used), row5 pre-scaled .5
                tts = smp.tile([32, NT], bf16, tag="tts")
                nc.scalar.activation(tts[0:6, :], psY[:], act.Tanh, scale=cf[:])
                # shuffle-broadcast rows 4 and 5 each to a base-0 tile
                uA = smp.tile([32, NT], bf16, tag="uA")
                nc.vector.stream_shuffle(uA[:], tts[:], [4] * 32)
                uB = smp.tile([32, NT], bf16, tag="uB")
                nc.vector.stream_shuffle(uB[:], tts[:], [5] * 32)
                # mask: invalid lanes get -20000 so exp underflows to 0
                uAm = smp.tile([1, NT], bf16, tag="uAm")
                nc.vector.tensor_tensor(
                    uAm[:], uA[0:1, :], m_sb[0:1, t * NT : (t + 1) * NT], alu.add
                )

                # v = masked_tanh(s1) * (tanh(s2/2) + 1) == 2*a ; e = exp(v/2)
                # (invalid: v <= -10000*(tanh+1) <= -5000 -> e == 0 exactly)
                v = smp.tile([1, NT], f32, tag="v")
                nc.vector.scalar_tensor_tensor(
                    v[:], uB[0:1, :], 1.0, uAm[:], alu.add, alu.mult
                )
                # e = exp(v/2); denominator partial = sum_n e (accum_out)
                e = smp.tile([32, NT], bf16, tag="e")
                nc.scalar.activation(
                    e[0:1, :], v[:], act.Exp, scale=0.5,
                    accum_out=sPP[0:1, t : t + 1],
                )
                # broadcast e to 4 partitions
                ebs = smp.tile([32, NT], bf16, tag="ebs")
                nc.vector.stream_shuffle(ebs[:], e[:], [0] * 32)

                # numerator partials: sYY[:, t] = sum_n psY[0:4]*e
                jY = smp.tile([4, NT], f32, tag="jY")
                nc.vector.scalar_tensor_tensor(
                    jY[:], psY[0:4, :], 1.0, ebs[0:4, :], alu.mult, alu.mult,
                    accum_out=sYY[0:4, t : t + 1],
                )

            # per-bag totals from per-tile partial columns
            redP = constp.tile([1, B], f32, tag="redP")
            nc.vector.tensor_reduce(
                redP[:],
                sPP[:].rearrange("p (b j) -> p b j", b=B),
                mybir.AxisListType.X,
                alu.add,
            )
            redY = constp.tile([4, B], f32, tag="redY")
            nc.vector.tensor_reduce(
                redY[:],
                sYY[:].rearrange("p (b j) -> p b j", b=B),
                mybir.AxisListType.X,
                alu.add,
            )

            # cc layout: flat[0:4] = s_b ; flat[4 + l*4 + b] = t_{b,l}
            cc_in = dramp.tile([1, 5 * B], f32, tag="cc_in")
            cc_out = dramp.tile([1, 5 * B], f32, tag="cc_out")
            nc.sync.dma_start(cc_in[0:1, 0:B], redP[:])
            nc.sync.dma_start(
                cc_in[0:1, B : 5 * B].rearrange("p (l b) -> l (p b)", l=4), redY[:]
            )
            nc.gpsimd.collective_compute(
                "AllReduce",
                alu.add,
                replica_groups=[list(range(NCORES))],
                ins=[cc_in[:].opt()],
                outs=[cc_out[:].opt()],
            )
            redsb = constp.tile([1, 5 * B], f32, tag="redsb")
            nc.sync.dma_start(redsb[:], cc_out[:])

            # out[b, l] = t_{b,l} / s_b
            rec = constp.tile([1, 5 * B], f32, tag="rec")
            nc.vector.reciprocal(rec[:], redsb[:])
            fin = constp.tile([1, B * L], f32, tag="fin")
            tv = redsb[:, B : 5 * B].rearrange("p (l b) -> p b l", l=L)
            rv = rec[:, 0:B].unsqueeze(2).broadcast_to([1, B, L])
            nc.vector.tensor_tensor(
                fin[:].rearrange("p (b l) -> p b l", b=B), tv, rv, alu.mult
            )
            nc.sync.dma_start(outp.rearrange("b l -> (b l)").unsqueeze(0), fin[:])

    nc.compile()
    return nc


def _get_nc():
    if "nc" not in _NC_CACHE:
        _NC_CACHE["nc"] = _build_nc()
    return _NC_CACHE["nc"]


def _make_in_maps(xs, valid, w1, w2, wf):
    import ml_dtypes

    validf = valid.astype(np.float32)
    xsz = (xs.astype(np.float32) * validf[..., None]).astype(np.float32)
    # [D, 6] = [wf.T | w1 | w2], packed per 128-chunk: (p, c*6+j) = W6[c*128+p, j]
    W6 = np.concatenate(
        [wf.astype(np.float32).T, w1.astype(np.float32), w2.astype(np.float32)],
        axis=1,
    )
    wsb = np.ascontiguousarray(
        W6.reshape(C8, 128, 6).transpose(1, 0, 2).reshape(128, C8 * 6)
    ).astype(ml_dtypes.bfloat16)

    cstf = np.array([[1.0]] * 5 + [[0.5]], np.float32)

    in_maps = []
    for c in range(NCORES):
        sh = xsz[:, c * NSH : (c + 1) * NSH, :].reshape(J, D)
        xt = sh.T  # [D, J]
        packed = np.ascontiguousarray(
            xt.reshape(C8, 128, T, NT).transpose(2, 1, 0, 3)
        ).reshape(T, 128, C8 * NT)
        mbc = np.ascontiguousarray(
            ((validf[:, c * NSH : (c + 1) * NSH] - 1.0) * 20000.0)
            .reshape(1, J)
            .astype(ml_dtypes.bfloat16)
        )
        in_maps.append({"xsp": packed, "wsb": wsb, "mb": mbc, "cstf": cstf})
    return in_maps


def _run(xs, valid, w1, w2, wf, trace=False, **kwargs):
    from concourse import bass_utils

    nc = _get_nc()
    in_maps = _make_in_maps(xs, valid, w1, w2, wf)
    res = bass_utils.run_bass_kernel_spmd(
        nc, in_maps, core_ids=list(range(NCORES)), trace=trace, **kwargs
    )
    return res


def kernel(xs, valid, w1, w2, wf):
    res = _run(xs, valid, w1, w2, wf, trace=False)
    return np.asarray(res.results[0]["out"]).astype(np.float32)


# revision 9
# speedup vs baseline: 1.7909x; 1.2674x over previous
"""ABMIL attention pooling on 8 TRN2 NeuronCores.

Algorithm (per bag b):
    a_n   = tanh(x_n . w1) * sigmoid(x_n . w2)            (gated attention score)
    att   = softmax over valid n of a                     (masked)
    out_b = sum_n att_n * (x_n . wf_l)                    (fold wf into the score matmul)

Key algebraic fold: out = (sum att_n x_n) @ wf.T == sum att_n (x_n @ wf.T),
so the only large compute is ONE matmul  y = xs @ [wf.T | w1 | w2]  ([N, 6])
and the pooling reduces tiny [N, 6] data.  Scores lie in (-1, 1) so no
softmax max-subtraction is needed: p = exp(a) * valid, out = sum(p*y)/sum(p).

sigmoid(x) = 0.5*(1 + tanh(x/2)) keeps all transcendentals in one ACT table
set (tanh + exp).

Per-core pipeline (memory-bound target: 32 MiB of xs @ ~355 GB/s ~ 95 us):
  - xs tiles stream in with an f32->bf16 cast-DMA (SWDGE, measured at line
    rate), so the PE runs bf16 at 1 col/cycle.
  - ONE 6-wide accumulation matmul group per 512-instance tile:
    psY[0:6, n] = [wf | w1 | w2].T @ x  (8 chunk matmuls over D=1024).
  - Engine ops need partition bases == 0 mod 32, so tanh runs on all 6 rows
    (rows 0..3 are don't-care) with per-partition scale [1,1,1,1,1,0.5], and
    the two score rows are extracted to base-0 PSUM tiles via K=6
    select-matmuls; e is broadcast to 4 partitions via a K=1 ones-matmul.
  - Per-bag accumulators (sum p, sum p*y) are separate base-0 tiles; final
    20-float stats are AllReduced across the 8 cores and out = t/s is
    computed on device (identical on every core).

Invalid instances are zeroed in the packed xs (host) so their y rows are 0;
the valid mask enters only the softmax denominator.
"""

import numpy as np

B, N, D, L = 4, 16384, 1024, 4
NCORES = 8
NSH = N // NCORES            # 2048 instances per bag per core
J = B * NSH                  # 8192 flattened rows per core
NT = 512                     # instances per compute tile (1 PSUM bank)
T = J // NT                  # 16 tiles
C8 = D // 128                # 8 contraction chunks of 128
TPB = NSH // NT              # tiles per bag = 4

_NC_CACHE = {}


def _build_nc():
    from concourse import bacc, mybir, tile

    dt = mybir.dt
    act = mybir.ActivationFunctionType
    alu = mybir.AluOpType
    f32 = dt.float32
    bf16 = dt.bfloat16

    nc = bacc.Bacc(
        "TRN2", target_bir_lowering=False, debug=False, num_devices=NCORES
    )

    xsp = nc.dram_tensor("xsp", [T, 128, C8 * NT], f32, kind="ExternalInput").ap()
    # [128, 48]: per chunk c, cols c*6..c*6+5 = [wf0..wf3, w1, w2]
    wsb = nc.dram_tensor("wsb", [128, C8 * 6], bf16, kind="ExternalInput").ap()
    # additive mask bias: 0 for valid, -20000 for invalid (pre-tanh-product)
    mb = nc.dram_tensor("mb", [1, J], bf16, kind="ExternalInput").ap()
    # cstf: per-partition tanh input scales [1,1,1,1,1,0.5]
    cstf = nc.dram_tensor("cstf", [6, 1], f32, kind="ExternalInput").ap()
    # per-core partial stats: [s_b (4) | t_{b,l} l-major (16)]
    outp = nc.dram_tensor("out", [1, 5 * B], f32, kind="ExternalOutput").ap()

    with tile.TileContext(nc) as tc:
        with (
            tc.tile_pool(name="const", bufs=1) as constp,
            tc.tile_pool(name="xs", bufs=6) as xpool,
            tc.tile_pool(name="psY", bufs=4, space="PSUM") as ypool,
            tc.tile_pool(name="sm", bufs=3) as smp,
        ):
            w_sb = constp.tile([128, C8 * 6], bf16, tag="w")
            nc.sync.dma_start(w_sb[:], wsb)
            m_sb = constp.tile([1, J], bf16, tag="m")
            nc.sync.dma_start(m_sb[:], mb)
            cf = constp.tile([6, 1], f32, tag="cf")
            nc.sync.dma_start(cf[:], cstf)
            sPP = constp.tile([1, T], f32, tag="sPP")
            sYY = constp.tile([4, T], f32, tag="sYY")

            for t in range(T):
                bg = t // TPB  # bag index for this tile
                xt = xpool.tile([128, C8 * NT], bf16, tag="xt")
                nc.gpsimd.dma_start(xt[:], xsp[t])  # f32 -> bf16 cast DMA

                # psY[0:6, n] = [wf | w1 | w2].T @ x  (accumulate over d)
                psY = ypool.tile([6, NT], f32, tag="y")
                for c in range(C8):
                    nc.tensor.matmul(
                        psY[:],
                        w_sb[:, c * 6 : (c + 1) * 6],
                        xt[:, c * NT : (c + 1) * NT],
                        start=(c == 0),
                        stop=(c == C8 - 1),
                    )

                # tanh of all 6 rows (only rows 4,5 used), row5 pre-scaled .5
                tts = smp.tile([32, NT], bf16, tag="tts")
                nc.scalar.activation(tts[0:6, :], psY[:], act.Tanh, scale=cf[:])
                # shuffle-broadcast rows 4 and 5 each to a base-0 tile
                uA = smp.tile([32, NT], bf16, tag="uA")
                nc.vector.stream_shuffle(uA[:], tts[:], [4] * 32)
                uB = smp.tile([32, NT], bf16, tag="uB")
                nc.vector.stream_shuffle(uB[:], tts[:], [5] * 32)
                # mask: invalid lanes get -20000 so exp underflows to 0
                uAm = smp.tile([1, NT], bf16, tag="uAm")
                nc.vector.tensor_tensor(
                    uAm[:], uA[0:1, :], m_sb[0:1, t * NT : (t + 1) * NT], alu.add
                )

                # v = masked_tanh(s1) * (tanh(s2/2) + 1) == 2*a ; e = exp(v/2)
                # (invalid: v <= -10000*(tanh+1) <= -5000 -> e == 0 exactly)
                v = smp.tile([1, NT], f32, tag="v")
                nc.vector.scalar_tensor_tensor(
                    v[:], uB[0:1, :], 1.0, uAm[:], alu.add, alu.mult
                )
                # e = exp(v/2); denominator partial = sum_n e (accum_out)
                e = smp.tile([32, NT], bf16, tag="e")
                nc.scalar.activation(
                    e[0:1, :], v[:], act.Exp, scale=0.5,
                    accum_out=sPP[0:1, t : t + 1],
                )
                # broadcast e to 4 partitions
                ebs = smp.tile([32, NT], bf16, tag="ebs")
                nc.vector.stream_shuffle(ebs[:], e[:], [0] * 32)

                # numerator partials: sYY[:, t] = sum_n psY[0:4]*e
                jY = smp.tile([4, NT], f32, tag="jY")
                nc.vector.scalar_tensor_tensor(
                    jY[:], psY[0:4, :], 1.0, ebs[0:4, :], alu.mult, alu.mult,
                    accum_out=sYY[0:4, t : t + 1],
                )

            # per-bag totals from per-tile partial columns
            redP = constp.tile([1, B], f32, tag="redP")
            nc.vector.tensor_reduce(
                redP[:],
                sPP[:].rearrange("p (b j) -> p b j", b=B),
                mybir.AxisListType.X,
                alu.add,
            )
            redY = constp.tile([4, B], f32, tag="redY")
            nc.vector.tensor_reduce(
                redY[:],
                sYY[:].rearrange("p (b j) -> p b j", b=B),
                mybir.AxisListType.X,
                alu.add,
            )

            # out layout: flat[0:4] = s_b ; flat[4 + l*4 + b] = t_{b,l}
            nc.sync.dma_start(outp[0:1, 0:B], redP[:])
            nc.sync.dma_start(
                outp[0:1, B : 5 * B].rearrange("p (l b) -> l (p b)", l=4), redY[:]
            )

    nc.compile()
    return nc


def _get_nc():
    if "nc" not in _NC_CACHE:
        _NC_CACHE["nc"] = _build_nc()
    return _NC_CACHE["nc"]


def _make_in_maps(xs, valid, w1, w2, wf):
    import ml_dtypes

    validf = valid.astype(np.float32)
    xsz = (xs.astype(np.float32) * validf[..., None]).astype(np.float32)
    # [D, 6] = [wf.T | w1 | w2], packed per 128-chunk: (p, c*6+j) = W6[c*128+p, j]
    W6 = np.concatenate(
        [wf.astype(np.float32).T, w1.astype(np.float32), w2.astype(np.float32)],
        axis=1,
    )
    wsb = np.ascontiguousarray(
        W6.reshape(C8, 128, 6).transpose(1, 0, 2).reshape(128, C8 * 6)
    ).astype(ml_dtypes.bfloat16)

    cstf = np.array([[1.0]] * 5 + [[0.5]], np.float32)

    in_maps = []
    for c in range(NCORES):
        sh = xsz[:, c * NSH : (c + 1) * NSH, :].reshape(J, D)
        xt = sh.T  # [D, J]
        packed = np.ascontiguousarray(
            xt.reshape(C8, 128, T, NT).transpose(2, 1, 0, 3)
        ).reshape(T, 128, C8 * NT)
        mbc = np.ascontiguousarray(
            ((validf[:, c * NSH : (c + 1) * NSH] - 1.0) * 20000.0)
            .reshape(1, J)
            .astype(ml_dtypes.bfloat16)
        )
        in_maps.append({"xsp": packed, "wsb": wsb, "mb": mbc, "cstf": cstf})
    return in_maps


def _run(xs, valid, w1, w2, wf, trace=False, **kwargs):
    from concourse import bass_utils

    nc = _get_nc()
    in_maps = _make_in_maps(xs, valid, w1, w2, wf)
    res = bass_utils.run_bass_kernel_spmd(
        nc, in_maps, core_ids=list(range(NCORES)), trace=trace, **kwargs
    )
    return res


def _combine(res):
    """Sum per-core partial stats (flash-style unshard) and finalize t/s."""
    tot = np.zeros(5 * B, np.float64)
    for c in range(NCORES):
        tot += np.asarray(res.results[c]["out"]).reshape(5 * B).astype(np.float64)
    s = tot[0:B]                                  # [b]
    t = tot[B:].reshape(L, B).T                   # [b, l]
    return (t / s[:, None]).astype(np.float32)


def kernel(xs, valid, w1, w2, wf):
    res = _run(xs, valid, w1, w2, wf, trace=False)
    return _combine(res)


# revision 10
# speedup vs baseline: 2.0990x; 1.1720x over previous
"""ABMIL attention pooling on 8 TRN2 NeuronCores.

Algorithm (per bag b):
    a_n   = tanh(x_n . w1) * sigmoid(x_n . w2)            (gated attention score)
    att   = softmax over valid n of a                     (masked)
    out_b = sum_n att_n * (x_n . wf_l)                    (fold wf into the score matmul)

Key algebraic fold: out = (sum att_n x_n) @ wf.T == sum att_n (x_n @ wf.T),
so the only large compute is ONE matmul  y = xs @ [wf.T | w1 | w2]  ([N, 6])
and the pooling reduces tiny [N, 6] data.  Scores lie in (-1, 1) so no
softmax max-subtraction is needed: p = exp(a) * valid, out = sum(p*y)/sum(p).

sigmoid(x) = 0.5*(1 + tanh(x/2)) keeps all transcendentals in one ACT table
set (tanh + exp).

Per-core pipeline (memory-bound target: 32 MiB of xs @ ~355 GB/s ~ 95 us):
  - xs tiles stream in with an f32->bf16 cast-DMA (SWDGE, measured at line
    rate), so the PE runs bf16 at 1 col/cycle.
  - ONE 6-wide accumulation matmul group per 512-instance tile:
    psY[0:6, n] = [wf | w1 | w2].T @ x  (8 chunk matmuls over D=1024).
  - Engine ops need partition bases == 0 mod 32, so tanh runs on all 6 rows
    (rows 0..3 are don't-care) with per-partition scale [1,1,1,1,1,0.5], and
    the two score rows are extracted to base-0 PSUM tiles via K=6
    select-matmuls; e is broadcast to 4 partitions via a K=1 ones-matmul.
  - Per-bag accumulators (sum p, sum p*y) are separate base-0 tiles; final
    20-float stats are AllReduced across the 8 cores and out = t/s is
    computed on device (identical on every core).

Invalid instances are zeroed in the packed xs (host) so their y rows are 0;
the valid mask enters only the softmax denominator.
"""

import numpy as np

B, N, D, L = 4, 16384, 1024, 4
NCORES = 8
NSH = N // NCORES            # 2048 instances per bag per core
J = B * NSH                  # 8192 flattened rows per core
NT = 512                     # instances per compute tile (1 PSUM bank)
T = J // NT                  # 16 tiles
C8 = D // 128                # 8 contraction chunks of 128
TPB = NSH // NT              # tiles per bag = 4

_NC_CACHE = {}


def _build_nc():
    from concourse import bacc, mybir, tile

    dt = mybir.dt
    act = mybir.ActivationFunctionType
    alu = mybir.AluOpType
    f32 = dt.float32
    bf16 = dt.bfloat16

    nc = bacc.Bacc(
        "TRN2", target_bir_lowering=False, debug=False, num_devices=NCORES
    )

    xsp = nc.dram_tensor("xsp", [T, 128, C8 * NT], f32, kind="ExternalInput").ap()
    # [128, 48]: per chunk c, cols c*6..c*6+5 = [wf0..wf3, w1, w2]
    wsb = nc.dram_tensor("wsb", [128, C8 * 6], bf16, kind="ExternalInput").ap()
    # additive mask bias: 0 for valid, -20000 for invalid (pre-tanh-product)
    mb = nc.dram_tensor("mb", [1, J], bf16, kind="ExternalInput").ap()
    # cstf: per-partition tanh input scales [1,1,1,1,1,0.5]
    cstf = nc.dram_tensor("cstf", [6, 1], f32, kind="ExternalInput").ap()
    # per-core partial stats: [s_b (4) | t_{b,l} l-major (16)]
    outp = nc.dram_tensor("out", [1, 5 * B], f32, kind="ExternalOutput").ap()

    with tile.TileContext(nc) as tc:
        with (
            tc.tile_pool(name="const", bufs=1) as constp,
            tc.tile_pool(name="xs", bufs=6) as xpool,
            tc.tile_pool(name="psY", bufs=4, space="PSUM") as ypool,
            tc.tile_pool(name="sm", bufs=3) as smp,
        ):
            w_sb = constp.tile([128, C8 * 6], bf16, tag="w")
            nc.sync.dma_start(w_sb[:], wsb)
            m_sb = constp.tile([1, J], bf16, tag="m")
            nc.sync.dma_start(m_sb[:], mb)
            cf = constp.tile([6, 1], f32, tag="cf")
            nc.sync.dma_start(cf[:], cstf)
            sPP = constp.tile([1, T], f32, tag="sPP")
            sYY = constp.tile([4, T], f32, tag="sYY")

            for tp in range(T // 2):
              # two tiles per iteration: their 16 matmuls sit back-to-back in
              # the PE FIFO, giving bursts long enough to get past the HAM
              # cold-clock window
              xts, psYs = [], []
              for t in (2 * tp, 2 * tp + 1):
                xt = xpool.tile([128, C8 * NT], bf16, tag=f"xt{t % 2}")
                nc.gpsimd.dma_start(xt[:], xsp[t])  # f32 -> bf16 cast DMA
                xts.append(xt)
              for t in (2 * tp, 2 * tp + 1):
                xt = xts[t % 2]
                psY = ypool.tile([6, NT], f32, tag=f"y{t % 2}")
                psYs.append(psY)
                for c in range(C8):
                    nc.tensor.matmul(
                        psY[:],
                        w_sb[:, c * 6 : (c + 1) * 6],
                        xt[:, c * NT : (c + 1) * NT],
                        start=(c == 0),
                        stop=(c == C8 - 1),
                    )
              for t in (2 * tp, 2 * tp + 1):
                bg = t // TPB  # bag index for this tile
                psY = psYs[t % 2]

                # tanh of all 6 rows (only rows 4,5 used), row5 pre-scaled .5
                tts = smp.tile([32, NT], bf16, tag="tts")
                nc.scalar.activation(tts[0:6, :], psY[:], act.Tanh, scale=cf[:])
                # shuffle-broadcast rows 4 and 5 each to a base-0 tile
                uA = smp.tile([32, NT], bf16, tag="uA")
                nc.vector.stream_shuffle(uA[:], tts[:], [4] * 32)
                uB = smp.tile([32, NT], bf16, tag="uB")
                nc.vector.stream_shuffle(uB[:], tts[:], [5] * 32)
                # mask: invalid lanes get -20000 so exp underflows to 0
                uAm = smp.tile([1, NT], bf16, tag="uAm")
                nc.vector.tensor_tensor(
                    uAm[:], uA[0:1, :], m_sb[0:1, t * NT : (t + 1) * NT], alu.add
                )

                # v = masked_tanh(s1) * (tanh(s2/2) + 1) == 2*a ; e = exp(v/2)
                # (invalid: v <= -10000*(tanh+1) <= -5000 -> e == 0 exactly)
                v = smp.tile([1, NT], f32, tag="v")
                nc.vector.scalar_tensor_tensor(
                    v[:], uB[0:1, :], 1.0, uAm[:], alu.add, alu.mult
                )
                # e = exp(v/2); denominator partial = sum_n e (accum_out)
                e = smp.tile([32, NT], bf16, tag="e")
                nc.scalar.activation(
                    e[0:1, :], v[:], act.Exp, scale=0.5,
                    accum_out=sPP[0:1, t : t + 1],
                )
                # broadcast e to 4 partitions
                ebs = smp.tile([32, NT], bf16, tag="ebs")
                nc.vector.stream_shuffle(ebs[:], e[:], [0] * 32)

                # numerator partials: sYY[:, t] = sum_n psY[0:4]*e
                jY = smp.tile([4, NT], f32, tag="jY")
                nc.vector.scalar_tensor_tensor(
                    jY[:], psY[0:4, :], 1.0, ebs[0:4, :], alu.mult, alu.mult,
                    accum_out=sYY[0:4, t : t + 1],
                )

            # per-bag totals from per-tile partial columns
            redP = constp.tile([1, B], f32, tag="redP")
            nc.vector.tensor_reduce(
                redP[:],
                sPP[:].rearrange("p (b j) -> p b j", b=B),
                mybir.AxisListType.X,
                alu.add,
            )
            redY = constp.tile([4, B], f32, tag="redY")
            nc.vector.tensor_reduce(
                redY[:],
                sYY[:].rearrange("p (b j) -> p b j", b=B),
                mybir.AxisListType.X,
                alu.add,
            )

            # out layout: flat[0:4] = s_b ; flat[4 + l*4 + b] = t_{b,l}
            nc.sync.dma_start(outp[0:1, 0:B], redP[:])
            nc.sync.dma_start(
                outp[0:1, B : 5 * B].rearrange("p (l b) -> l (p b)", l=4), redY[:]
            )

    nc.compile()
    return nc


def _get_nc():
    if "nc" not in _NC_CACHE:
        _NC_CACHE["nc"] = _build_nc()
    return _NC_CACHE["nc"]


def _make_in_maps(xs, valid, w1, w2, wf):
    import ml_dtypes

    validf = valid.astype(np.float32)
    xsz = (xs.astype(np.float32) * validf[..., None]).astype(np.float32)
    # [D, 6] = [wf.T | w1 | w2], packed per 128-chunk: (p, c*6+j) = W6[c*128+p, j]
    W6 = np.concatenate(
        [wf.astype(np.float32).T, w1.astype(np.float32), w2.astype(np.float32)],
        axis=1,
    )
    wsb = np.ascontiguousarray(
        W6.reshape(C8, 128, 6).transpose(1, 0, 2).reshape(128, C8 * 6)
    ).astype(ml_dtypes.bfloat16)

    cstf = np.array([[1.0]] * 5 + [[0.5]], np.float32)

    in_maps = []
    for c in range(NCORES):
        sh = xsz[:, c * NSH : (c + 1) * NSH, :].reshape(J, D)
        xt = sh.T  # [D, J]
        packed = np.ascontiguousarray(
            xt.reshape(C8, 128, T, NT).transpose(2, 1, 0, 3)
        ).reshape(T, 128, C8 * NT)
        mbc = np.ascontiguousarray(
            ((validf[:, c * NSH : (c + 1) * NSH] - 1.0) * 20000.0)
            .reshape(1, J)
            .astype(ml_dtypes.bfloat16)
        )
        in_maps.append({"xsp": packed, "wsb": wsb, "mb": mbc, "cstf": cstf})
    return in_maps


def _run(xs, valid, w1, w2, wf, trace=False, **kwargs):
    from concourse import bass_utils

    nc = _get_nc()
    in_maps = _make_in_maps(xs, valid, w1, w2, wf)
    res = bass_utils.run_bass_kernel_spmd(
        nc, in_maps, core_ids=list(range(NCORES)), trace=trace, **kwargs
    )
    return res


def _combine(res):
    """Sum per-core partial stats (flash-style unshard) and finalize t/s."""
    tot = np.zeros(5 * B, np.float64)
    for c in range(NCORES):
        tot += np.asarray(res.results[c]["out"]).reshape(5 * B).astype(np.float64)
    s = tot[0:B]                                  # [b]
    t = tot[B:].reshape(L, B).T                   # [b, l]
    return (t / s[:, None]).astype(np.float32)


def kernel(xs, valid, w1, w2, wf):
    res = _run(xs, valid, w1, w2, wf, trace=False)
    return _combine(res)


# revision 11
# speedup vs baseline: 2.1961x; 1.0463x over previous
"""ABMIL attention pooling on 8 TRN2 NeuronCores.

Algorithm (per bag b):
    a_n   = tanh(x_n . w1) * sigmoid(x_n . w2)            (gated attention score)
    att   = softmax over valid n of a                     (masked)
    out_b = sum_n att_n * (x_n . wf_l)                    (fold wf into the score matmul)

Key algebraic fold: out = (sum att_n x_n) @ wf.T == sum att_n (x_n @ wf.T),
so the only large compute is ONE matmul  y = xs @ [wf.T | w1 | w2]  ([N, 6])
and the pooling reduces tiny [N, 6] data.  Scores lie in (-1, 1) so no
softmax max-subtraction is needed: p = exp(a) * valid, out = sum(p*y)/sum(p).

sigmoid(x) = 0.5*(1 + tanh(x/2)) keeps all transcendentals in one ACT table
set (tanh + exp).

Per-core pipeline (memory-bound target: 32 MiB of xs @ ~355 GB/s ~ 95 us):
  - xs tiles stream in with an f32->bf16 cast-DMA (SWDGE, measured at line
    rate), so the PE runs bf16 at 1 col/cycle.
  - ONE 6-wide accumulation matmul group per 512-instance tile:
    psY[0:6, n] = [wf | w1 | w2].T @ x  (8 chunk matmuls over D=1024).
  - Engine ops need partition bases == 0 mod 32, so tanh runs on all 6 rows
    (rows 0..3 are don't-care) with per-partition scale [1,1,1,1,1,0.5], and
    the two score rows are extracted to base-0 PSUM tiles via K=6
    select-matmuls; e is broadcast to 4 partitions via a K=1 ones-matmul.
  - Per-bag accumulators (sum p, sum p*y) are separate base-0 tiles; final
    20-float stats are AllReduced across the 8 cores and out = t/s is
    computed on device (identical on every core).

Invalid instances are zeroed in the packed xs (host) so their y rows are 0;
the valid mask enters only the softmax denominator.
"""

import numpy as np

B, N, D, L = 4, 16384, 1024, 4
NCORES = 8
NSH = N // NCORES            # 2048 instances per bag per core
J = B * NSH                  # 8192 flattened rows per core
NT = 512                     # instances per compute tile (1 PSUM bank)
T = J // NT                  # 16 tiles
C8 = D // 128                # 8 contraction chunks of 128
TPB = NSH // NT              # tiles per bag = 4

_NC_CACHE = {}


def _build_nc():
    from concourse import bacc, mybir, tile

    dt = mybir.dt
    act = mybir.ActivationFunctionType
    alu = mybir.AluOpType
    f32 = dt.float32
    bf16 = dt.bfloat16

    nc = bacc.Bacc(
        "TRN2", target_bir_lowering=False, debug=False, num_devices=NCORES
    )

    xsp = nc.dram_tensor("xsp", [T, 128, C8 * NT], f32, kind="ExternalInput").ap()
    # [128, 48]: per chunk c, cols c*6..c*6+5 = [wf0..wf3, w1, w2]
    wsb = nc.dram_tensor("wsb", [128, C8 * 6], bf16, kind="ExternalInput").ap()
    # cstf: per-partition tanh input scales [1,1,1,1,1,0.5]
    cstf = nc.dram_tensor("cstf", [6, 1], f32, kind="ExternalInput").ap()
    # per-core partial stats: [s_b (4) | t_{b,l} l-major (16)]
    outp = nc.dram_tensor("out", [1, 5 * B], f32, kind="ExternalOutput").ap()

    with tile.TileContext(nc) as tc:
        with (
            tc.tile_pool(name="const", bufs=1) as constp,
            tc.tile_pool(name="xs", bufs=6) as xpool,
            tc.tile_pool(name="psY", bufs=4, space="PSUM") as ypool,
            tc.tile_pool(name="sm", bufs=3) as smp,
        ):
            w_sb = constp.tile([128, C8 * 6], bf16, tag="w")
            nc.sync.dma_start(w_sb[:], wsb)
            cf = constp.tile([6, 1], f32, tag="cf")
            nc.sync.dma_start(cf[:], cstf)
            sPP = constp.tile([1, T], f32, tag="sPP")
            sYY = constp.tile([4, T], f32, tag="sYY")
            redP = constp.tile([1, B], f32, tag="redP")
            redY = constp.tile([4, B], f32, tag="redY")

            for tp in range(T // 2):
              # two tiles per iteration: their 16 matmuls sit back-to-back in
              # the PE FIFO, giving bursts long enough to get past the HAM
              # cold-clock window
              xts, psYs = [], []
              for t in (2 * tp, 2 * tp + 1):
                xt = xpool.tile([128, C8 * NT], bf16, tag=f"xt{t % 2}")
                nc.gpsimd.dma_start(xt[:], xsp[t])  # f32 -> bf16 cast DMA
                xts.append(xt)
              for t in (2 * tp, 2 * tp + 1):
                xt = xts[t % 2]
                psY = ypool.tile([6, NT], f32, tag=f"y{t % 2}")
                psYs.append(psY)
                for c in range(C8):
                    nc.tensor.matmul(
                        psY[:],
                        w_sb[:, c * 6 : (c + 1) * 6],
                        xt[:, c * NT : (c + 1) * NT],
                        start=(c == 0),
                        stop=(c == C8 - 1),
                    )
              for t in (2 * tp, 2 * tp + 1):
                bg = t // TPB  # bag index for this tile
                psY = psYs[t % 2]

                # tanh of all 6 rows (only rows 4,5 used), row5 pre-scaled .5
                tts = smp.tile([32, NT], bf16, tag="tts")
                nc.scalar.activation(tts[0:6, :], psY[:], act.Tanh, scale=cf[:])
                # shuffle-broadcast rows 4 and 5 each to a base-0 tile
                uA = smp.tile([32, NT], bf16, tag="uA")
                nc.vector.stream_shuffle(uA[:], tts[:], [4] * 32)
                uB = smp.tile([32, NT], bf16, tag="uB")
                nc.vector.stream_shuffle(uB[:], tts[:], [5] * 32)
                # v = tanh(s1) * (tanh(s2/2) + 1) == 2*a ; e = exp(v/2)
                # invalid instances have zeroed xs -> e = exp(0) = 1 exactly;
                # the host subtracts the per-bag invalid count from sum(e)
                v = smp.tile([1, NT], f32, tag="v")
                nc.vector.scalar_tensor_tensor(
                    v[:], uB[0:1, :], 1.0, uA[0:1, :], alu.add, alu.mult
                )
                # e = exp(v/2); denominator partial = sum_n e (accum_out)
                e = smp.tile([32, NT], bf16, tag="e")
                nc.scalar.activation(
                    e[0:1, :], v[:], act.Exp, scale=0.5,
                    accum_out=sPP[0:1, t : t + 1],
                )
                # broadcast e to 4 partitions
                ebs = smp.tile([32, NT], bf16, tag="ebs")
                nc.vector.stream_shuffle(ebs[:], e[:], [0] * 32)

                # numerator partials: sYY[:, t] = sum_n psY[0:4]*e
                jY = smp.tile([4, NT], f32, tag="jY")
                nc.vector.scalar_tensor_tensor(
                    jY[:], psY[0:4, :], 1.0, ebs[0:4, :], alu.mult, alu.mult,
                    accum_out=sYY[0:4, t : t + 1],
                )
                if t % TPB == TPB - 1:
                    # bag bg complete: fold its 4 partial columns immediately
                    nc.vector.tensor_reduce(
                        redP[0:1, bg : bg + 1],
                        sPP[0:1, bg * TPB : (bg + 1) * TPB],
                        mybir.AxisListType.X,
                        alu.add,
                    )
                    nc.vector.tensor_reduce(
                        redY[0:4, bg : bg + 1],
                        sYY[0:4, bg * TPB : (bg + 1) * TPB],
                        mybir.AxisListType.X,
                        alu.add,
                    )

            # out layout: flat[0:4] = s_b ; flat[4 + l*4 + b] = t_{b,l}
            nc.sync.dma_start(outp[0:1, 0:B], redP[:])
            nc.sync.dma_start(
                outp[0:1, B : 5 * B].rearrange("p (l b) -> l (p b)", l=4), redY[:]
            )

    nc.compile()
    return nc


def _get_nc():
    if "nc" not in _NC_CACHE:
        _NC_CACHE["nc"] = _build_nc()
    return _NC_CACHE["nc"]


def _make_in_maps(xs, valid, w1, w2, wf):
    import ml_dtypes

    validf = valid.astype(np.float32)
    xsz = (xs.astype(np.float32) * validf[..., None]).astype(np.float32)
    # [D, 6] = [wf.T | w1 | w2], packed per 128-chunk: (p, c*6+j) = W6[c*128+p, j]
    W6 = np.concatenate(
        [wf.astype(np.float32).T, w1.astype(np.float32), w2.astype(np.float32)],
        axis=1,
    )
    wsb = np.ascontiguousarray(
        W6.reshape(C8, 128, 6).transpose(1, 0, 2).reshape(128, C8 * 6)
    ).astype(ml_dtypes.bfloat16)

    cstf = np.array([[1.0]] * 5 + [[0.5]], np.float32)

    in_maps = []
    for c in range(NCORES):
        sh = xsz[:, c * NSH : (c + 1) * NSH, :].reshape(J, D)
        xt = sh.T  # [D, J]
        packed = np.ascontiguousarray(
            xt.reshape(C8, 128, T, NT).transpose(2, 1, 0, 3)
        ).reshape(T, 128, C8 * NT)
        in_maps.append({"xsp": packed, "wsb": wsb, "cstf": cstf})
    return in_maps


def _run(xs, valid, w1, w2, wf, trace=False, **kwargs):
    from concourse import bass_utils

    nc = _get_nc()
    in_maps = _make_in_maps(xs, valid, w1, w2, wf)
    res = bass_utils.run_bass_kernel_spmd(
        nc, in_maps, core_ids=list(range(NCORES)), trace=trace, **kwargs
    )
    return res


def _combine(res, valid):
    """Sum per-core partial stats (flash-style unshard) and finalize t/s.

    Invalid instances were zeroed on the device input, so each contributes
    exp(0) = 1 to the denominator partials; subtract their count here.
    """
    tot = np.zeros(5 * B, np.float64)
    for c in range(NCORES):
        tot += np.asarray(res.results[c]["out"]).reshape(5 * B).astype(np.float64)
    n_invalid = (~valid.astype(bool)).sum(axis=1).astype(np.float64)  # [b]
    s = tot[0:B] - n_invalid                      # [b]
    t = tot[B:].reshape(L, B).T                   # [b, l]
    return (t / s[:, None]).astype(np.float32)


def kernel(xs, valid, w1, w2, wf):
    res = _run(xs, valid, w1, w2, wf, trace=False)
    return _combine(res, np.asarray(valid))
